# revision 7
# baseline (speedup 1.0000x reference)
"""ChildSumTreeLSTM on a complete binary tree (N=8191), 8-core Trainium2.

Strategy: the heap-ordered complete tree = 7 top nodes + 8 independent
1023-node subtrees. Each NeuronCore gets one subtree (tree-level
parallelism), computes the batched x-projections (emb lookup done on host,
projections as dense matmuls on the PE) and a level-synchronous scan
(leaves -> subtree root) with everything feature-major [256 feats x nodes].
One 16KB AllGather shares the 8 subtree roots; the top 3 levels are
computed redundantly on every core. Output read from core 0.
"""

import numpy as np

import concourse.bass as bass
import concourse.tile as tile
from concourse import mybir
from concourse.bass_utils import run_bass_kernel_spmd

F32 = mybir.dt.float32
AFT = mybir.ActivationFunctionType

N_NODES = 8191
D = 256
M = 256
NCOL = 1152  # 1023 subtree cols + 7 top cols + pad
SUB_LEVELS = 10  # subtree: 512 leaves ... 1 root
USE_F32R = False  # flip to use TF32-like fast fp32 matmuls


def _split_excess_waits(nc, max_waits=1):
    """walrus in this container allows only 1 sync-wait per instruction.

    Tile can attach several; hoist the extras onto injected same-engine NOPs
    immediately preceding the instruction (same blocking semantics)."""
    k = 0
    for f in nc.m.functions:
        for bb in f.blocks:
            out = []
            changed = False
            for ins in bb.instructions:
                si = ins.sync_info
                w = list(si.on_wait) if si and si.on_wait else []
                if len(w) > max_waits:
                    hoist, keep = w[:-max_waits], w[-max_waits:]
                    for sw in hoist:
                        nop = mybir.InstNoOp(name=f"whoist{k}", ins=[], outs=[])
                        k += 1
                        nop.engine = ins.engine
                        nop.sync_info = mybir.SyncInfo(on_wait=[sw], on_update=[])
                        out.append(nop)
                    si.on_wait = keep
                    changed = True
                out.append(ins)
            if changed:
                bb.instructions = out


def _mmcast(ap):
    return ap.bitcast(mybir.dt.float32r) if USE_F32R else ap


def _build_module():
    nc = bass.Bass(num_devices=8)

    xT = nc.dram_tensor("xT", [D, NCOL], F32, kind="ExternalInput")
    wcT = nc.dram_tensor("wcT", [D, 1024], F32, kind="ExternalInput")
    wiouhT = nc.dram_tensor("wiouhT", [M, 768], F32, kind="ExternalInput")
    wfhT = nc.dram_tensor("wfhT", [M, 256], F32, kind="ExternalInput")
    ident = nc.dram_tensor("ident", [128, 128], F32, kind="ExternalInput")
    b_iou_int = nc.dram_tensor("b_iou_int", [128, 6], F32, kind="ExternalInput")
    b_iou_leaf = nc.dram_tensor("b_iou_leaf", [128, 6], F32, kind="ExternalInput")
    b_f_int = nc.dram_tensor("b_f_int", [128, 2], F32, kind="ExternalInput")
    b_f_leaf = nc.dram_tensor("b_f_leaf", [128, 2], F32, kind="ExternalInput")
    out = nc.dram_tensor("out", [512, 1], F32, kind="ExternalOutput")

    agin = nc.dram_tensor("agin", [512, 1], F32)
    agout = nc.dram_tensor("agout", [4096, 1], F32, addr_space="Shared")

    with tile.TileContext(nc) as tc:
        with (
            tc.tile_pool(name="consts", bufs=1) as consts,
            tc.tile_pool(name="tmps", bufs=2) as tmps,
            tc.tile_pool(name="scan_psum", bufs=1, space="PSUM") as spsum,
        ):
            # ---- resident SBUF tensors ----
            sb_xT = []
            for kt in range(2):
                t = consts.tile([128, NCOL], F32, tag=f"xT{kt}")
                nc.gpsimd.dma_start(out=t[:], in_=xT[128 * kt : 128 * (kt + 1), :])
                sb_xT.append(t)
            sb_wcT = []
            for kt in range(2):
                t = consts.tile([128, 1024], F32, tag=f"wcT{kt}")
                nc.gpsimd.dma_start(out=t[:], in_=wcT[128 * kt : 128 * (kt + 1), :])
                sb_wcT.append(t)
            sb_wiouhT = []
            for kt in range(2):
                t = consts.tile([128, 768], F32, tag=f"wiouhT{kt}")
                nc.gpsimd.dma_start(out=t[:], in_=wiouhT[128 * kt : 128 * (kt + 1), :])
                sb_wiouhT.append(t)
            sb_wfhT = []
            for kt in range(2):
                t = consts.tile([128, 256], F32, tag=f"wfhT{kt}")
                nc.gpsimd.dma_start(out=t[:], in_=wfhT[128 * kt : 128 * (kt + 1), :])
                sb_wfhT.append(t)
            sb_id = consts.tile([128, 128], F32, tag="ident")
            nc.gpsimd.dma_start(out=sb_id[:], in_=ident[:])
            sb_biou_i = consts.tile([128, 6], F32, tag="biou_i")
            nc.gpsimd.dma_start(out=sb_biou_i[:], in_=b_iou_int[:])
            sb_biou_l = consts.tile([128, 6], F32, tag="biou_l")
            nc.gpsimd.dma_start(out=sb_biou_l[:], in_=b_iou_leaf[:])
            sb_bf_i = consts.tile([128, 2], F32, tag="bf_i")
            nc.gpsimd.dma_start(out=sb_bf_i[:], in_=b_f_int[:])
            sb_bf_l = consts.tile([128, 2], F32, tag="bf_l")
            nc.gpsimd.dma_start(out=sb_bf_l[:], in_=b_f_leaf[:])

            # IOUXFX[F][p, c]: feature 128F+p for node col c.
            # F 0..1 = i, 2..3 = o, 4..5 = u, 6..7 = fx  (no biases folded)
            IOUXFX = [consts.tile([128, NCOL], F32, tag=f"iouxfx{F}", name=f"iouxfx{F}") for F in range(8)]
            # c/h state for the subtree, cols = local heap index 0..1022
            C = [consts.tile([128, 1024], F32, tag=f"C{h}", name=f"C{h}") for h in range(2)]
            H = [consts.tile([128, 1024], F32, tag=f"H{h}", name=f"H{h}") for h in range(2)]

            # ---- phase 1: x-projections  IOUXFX = Wc @ x ----
            psum_tags = [f"iou{F}" for F in range(6)] + ["f0", "f1"]
            pre_i = 0
            if True:
                for F in range(8):
                    for c0, cw in ((0, 512), (512, 512), (1024, NCOL - 1024)):
                        ps = spsum.tile([128, 512], F32, tag=psum_tags[pre_i % 8], name=f"pre_ps{pre_i}")
                        pre_i += 1
                        for kt in range(2):
                            nc.tensor.matmul(
                                ps[:, :cw],
                                _mmcast(sb_wcT[kt][:, 128 * F : 128 * (F + 1)]),
                                _mmcast(sb_xT[kt][:, c0 : c0 + cw]),
                                start=(kt == 0),
                                stop=(kt == 1),
                            )
                        nc.vector.tensor_copy(IOUXFX[F][:, c0 : c0 + cw], ps[:, :cw])

            # ---- phase 2: leaves (local heap 511..1022 -> cols [511:1023)) ----
            a, b = 511, 1023
            for h in range(2):
                sig_i = tmps.tile([128, 512], F32, tag="sig_i")
                nc.scalar.activation(
                    sig_i[:], IOUXFX[0 + h][:, a:b], AFT.Sigmoid,
                    bias=sb_biou_l[:, 0 + h : 1 + h],
                )
                sig_o = tmps.tile([128, 512], F32, tag="sig_o")
                nc.scalar.activation(
                    sig_o[:], IOUXFX[2 + h][:, a:b], AFT.Sigmoid,
                    bias=sb_biou_l[:, 2 + h : 3 + h],
                )
                tanh_u = tmps.tile([128, 512], F32, tag="tanh_u")
                nc.scalar.activation(
                    tanh_u[:], IOUXFX[4 + h][:, a:b], AFT.Tanh,
                    bias=sb_biou_l[:, 4 + h : 5 + h],
                )
                fc = tmps.tile([128, 512], F32, tag="fc")
                nc.scalar.activation(
                    fc[:], IOUXFX[6 + h][:, a:b], AFT.Sigmoid,
                    bias=sb_bf_l[:, h : h + 1],
                )
                iu = tmps.tile([128, 512], F32, tag="iu")
                nc.vector.tensor_mul(iu[:], sig_i[:], tanh_u[:])
                nc.vector.tensor_add(C[h][:, a:b], iu[:], fc[:])
                tanh_c = tmps.tile([128, 512], F32, tag="tanh_c")
                nc.scalar.activation(tanh_c[:], C[h][:, a:b], AFT.Tanh)
                nc.vector.tensor_mul(H[h][:, a:b], sig_o[:], tanh_c[:])

            # ---- internal level routine (feature-major) ----
            def internal_level(n, ioux_lo, childC, childH, Cout, Hout):
                # childC/childH: per h/kt APs [128, 2n] (child cols, heap order)
                # Cout/Hout: per h APs [128, n]
                hs = []
                for kt in range(2):
                    t = tmps.tile([128, max(n, 1)], F32, tag="hs")
                    nc.vector.tensor_add(
                        t[:, :n], childH[kt][:, 0::2], childH[kt][:, 1::2]
                    )
                    hs.append(t)
                ps_iou = []
                for F in range(6):
                    ps = spsum.tile([128, 512], F32, tag=f"iou{F}")
                    for kt in range(2):
                        nc.tensor.matmul(
                            ps[:, :n],
                            _mmcast(sb_wiouhT[kt][:, 128 * F : 128 * (F + 1)]),
                            _mmcast(hs[kt][:, :n]),
                            start=(kt == 0),
                            stop=False,
                        )
                    nc.tensor.matmul(
                        ps[:, :n],
                        _mmcast(sb_id[:]),
                        _mmcast(IOUXFX[F][:, ioux_lo : ioux_lo + n]),
                        start=False,
                        stop=True,
                    )
                    ps_iou.append(ps)
                ps_f = []
                for h in range(2):
                    ps = spsum.tile([128, 512], F32, tag=f"f{h}")
                    for kt in range(2):
                        nc.tensor.matmul(
                            ps[:, : 2 * n],
                            _mmcast(sb_wfhT[kt][:, 128 * h : 128 * (h + 1)]),
                            _mmcast(childH[kt]),
                            start=(kt == 0),
                            stop=False,
                        )
                    # += fx[parent] duplicated onto both child slots
                    fxdup = (
                        IOUXFX[6 + h][:, ioux_lo : ioux_lo + n]
                        .unsqueeze(2)
                        .broadcast_to([128, n, 2])
                    )
                    nc.tensor.matmul(
                        ps[:, : 2 * n], _mmcast(sb_id[:]), _mmcast(fxdup),
                        start=False, stop=True,
                    )
                    ps_f.append(ps)
                for h in range(2):
                    sig_i = tmps.tile([128, max(n, 1)], F32, tag="sig_i")
                    nc.scalar.activation(
                        sig_i[:, :n], ps_iou[0 + h][:, :n], AFT.Sigmoid,
                        bias=sb_biou_i[:, 0 + h : 1 + h],
                    )
                    sig_o = tmps.tile([128, max(n, 1)], F32, tag="sig_o")
                    nc.scalar.activation(
                        sig_o[:, :n], ps_iou[2 + h][:, :n], AFT.Sigmoid,
                        bias=sb_biou_i[:, 2 + h : 3 + h],
                    )
                    tanh_u = tmps.tile([128, max(n, 1)], F32, tag="tanh_u")
                    nc.scalar.activation(
                        tanh_u[:, :n], ps_iou[4 + h][:, :n], AFT.Tanh,
                        bias=sb_biou_i[:, 4 + h : 5 + h],
                    )
                    f = tmps.tile([128, max(2 * n, 1)], F32, tag="f")
                    nc.scalar.activation(
                        f[:, : 2 * n], ps_f[h][:, : 2 * n], AFT.Sigmoid,
                        bias=sb_bf_i[:, h : h + 1],
                    )
                    g = tmps.tile([128, max(2 * n, 1)], F32, tag="g")
                    nc.vector.tensor_mul(g[:, : 2 * n], f[:, : 2 * n], childC[h])
                    fc = tmps.tile([128, max(n, 1)], F32, tag="fc")
                    nc.vector.tensor_add(fc[:, :n], g[:, 0 : 2 * n : 2], g[:, 1 : 2 * n : 2])
                    iu = tmps.tile([128, max(n, 1)], F32, tag="iu")
                    nc.vector.tensor_mul(iu[:, :n], sig_i[:, :n], tanh_u[:, :n])
                    nc.vector.tensor_add(Cout[h], iu[:, :n], fc[:, :n])
                    tanh_c = tmps.tile([128, max(n, 1)], F32, tag="tanh_c")
                    nc.scalar.activation(tanh_c[:, :n], Cout[h], AFT.Tanh)
                    nc.vector.tensor_mul(Hout[h], sig_o[:, :n], tanh_c[:, :n])

            # ---- phase 3: subtree internal levels (l = 8 .. 0) ----
            for l in range(8, -1, -1):
                n = 1 << l
                a, b = n - 1, 2 * n - 1
                a2, b2 = 2 * n - 1, 4 * n - 1
                internal_level(
                    n,
                    a,
                    [C[h][:, a2:b2] for h in range(2)],
                    [H[kt][:, a2:b2] for kt in range(2)],
                    [C[h][:, a:b] for h in range(2)],
                    [H[h][:, a:b] for h in range(2)],
                )

            # ---- phase 4: AllGather subtree roots ----
            for h in range(2):
                nc.gpsimd.dma_start(
                    out=agin[128 * h : 128 * (h + 1), :], in_=C[h][:, 0:1]
                )
                nc.gpsimd.dma_start(
                    out=agin[256 + 128 * h : 256 + 128 * (h + 1), :], in_=H[h][:, 0:1]
                )
            nc.gpsimd.collective_compute(
                "AllGather",
                mybir.AluOpType.bypass,
                replica_groups=[list(range(8))],
                ins=[agin[:]],
                outs=[agout[:]],
            )
            C3 = [consts.tile([128, 8], F32, tag=f"C3_{h}", name=f"C3_{h}") for h in range(2)]
            H3 = [consts.tile([128, 8], F32, tag=f"H3_{h}", name=f"H3_{h}") for h in range(2)]
            for h in range(2):
                nc.gpsimd.dma_start(
                    out=C3[h][:],
                    in_=bass.AP(tensor=agout, offset=128 * h, ap=[[1, 128], [512, 8]]),
                )
                nc.gpsimd.dma_start(
                    out=H3[h][:],
                    in_=bass.AP(
                        tensor=agout, offset=256 + 128 * h, ap=[[1, 128], [512, 8]]
                    ),
                )

            # ---- phase 5: top 3 levels (redundant on every core) ----
            C2 = [consts.tile([128, 4], F32, tag=f"C2_{h}", name=f"C2_{h}") for h in range(2)]
            H2 = [consts.tile([128, 4], F32, tag=f"H2_{h}", name=f"H2_{h}") for h in range(2)]
            internal_level(
                4, 1026,
                [C3[h][:] for h in range(2)],
                [H3[kt][:] for kt in range(2)],
                [C2[h][:] for h in range(2)],
                [H2[h][:] for h in range(2)],
            )
            C1 = [consts.tile([128, 2], F32, tag=f"C1_{h}", name=f"C1_{h}") for h in range(2)]
            H1 = [consts.tile([128, 2], F32, tag=f"H1_{h}", name=f"H1_{h}") for h in range(2)]
            internal_level(
                2, 1024,
                [C2[h][:] for h in range(2)],
                [H2[kt][:] for kt in range(2)],
                [C1[h][:] for h in range(2)],
                [H1[h][:] for h in range(2)],
            )
            C0 = [consts.tile([128, 1], F32, tag=f"C0_{h}", name=f"C0_{h}") for h in range(2)]
            H0 = [consts.tile([128, 1], F32, tag=f"H0_{h}", name=f"H0_{h}") for h in range(2)]
            internal_level(
                1, 1023,
                [C1[h][:] for h in range(2)],
                [H1[kt][:] for kt in range(2)],
                [C0[h][:] for h in range(2)],
                [H0[h][:] for h in range(2)],
            )
            for h in range(2):
                nc.gpsimd.dma_start(out=out[128 * h : 128 * (h + 1), :], in_=C0[h][:])
                nc.gpsimd.dma_start(
                    out=out[256 + 128 * h : 256 + 128 * (h + 1), :], in_=H0[h][:]
                )
    _split_excess_waits(nc)
    return nc


_NC_CACHE = None


def _get_module():
    global _NC_CACHE
    if _NC_CACHE is None:
        _NC_CACHE = _build_module()
    return _NC_CACHE


def _expected_children():
    j = (N_NODES - 1) - np.arange(N_NODES)
    internal = (2 * j + 1) < N_NODES
    ch0 = (N_NODES - 1) - (2 * j + 1)
    ch1 = (N_NODES - 1) - (2 * j + 2)
    children = np.stack(
        [np.where(internal, ch0, 0), np.where(internal, ch1, 0)], axis=1
    ).astype(np.int32)
    mask = np.stack([internal, internal], axis=1)
    return children, mask


def _reference_numpy(emb, W_ioux, b_ioux, W_iouh, b_iouh, W_fx, b_fx, W_fh, b_fh,
                     ops, children, child_mask):
    # generic fallback (matches reference.py) for unexpected tree structure
    def sigmoid(v):
        return 1.0 / (1.0 + np.exp(-v))

    N = ops.shape[0]
    Md = W_fh.shape[0]
    x = emb[ops]
    iou_x = x @ W_ioux.T + b_ioux
    fx_all = x @ W_fx.T + b_fx
    ones = np.ones((Md,), np.float32)
    leaf_fh = ones @ W_fh.T + b_fh
    maskf = child_mask.astype(np.float32)
    c_arr = np.zeros((N, Md), np.float32)
    h_arr = np.zeros((N, Md), np.float32)
    for t in range(N):
        idx = children[t]
        m = maskf[t][:, None]
        ch_c = c_arr[idx] * m
        ch_h = h_arr[idx] * m
        is_leaf = maskf[t].sum() == 0
        h_sum = ones if is_leaf else ch_h.sum(0)
        iou = iou_x[t] + h_sum @ W_iouh.T + b_iouh
        i, o, u = np.split(iou, 3)
        i, o, u = sigmoid(i), sigmoid(o), np.tanh(u)
        f = sigmoid(ch_h @ W_fh.T + b_fh + fx_all[t])
        fc_int = (f * ch_c).sum(0)
        fc_leaf = sigmoid(leaf_fh + fx_all[t])
        fc = fc_leaf if is_leaf else fc_int
        c = i * u + fc
        h = o * np.tanh(c)
        c_arr[t] = c
        h_arr[t] = h
    return np.stack([c_arr[N - 1], h_arr[N - 1]])


def _col_index_for_core(k):
    # columns 0..1022: subtree-local heap order; 1023..1029: global heap 0..6
    idx = np.zeros(NCOL, np.int64)
    for l in range(SUB_LEVELS):
        n = 1 << l
        lo = n - 1
        g0 = (1 << (3 + l)) - 1 + k * n
        idx[lo : lo + n] = g0 + np.arange(n)
    idx[1023:1030] = np.arange(7)
    return idx


def kernel(**inputs):
    emb = np.asarray(inputs["emb"], np.float32)
    W_ioux = np.asarray(inputs["W_ioux"], np.float32)
    b_ioux = np.asarray(inputs["b_ioux"], np.float32)
    W_iouh = np.asarray(inputs["W_iouh"], np.float32)
    b_iouh = np.asarray(inputs["b_iouh"], np.float32)
    W_fx = np.asarray(inputs["W_fx"], np.float32)
    b_fx = np.asarray(inputs["b_fx"], np.float32)
    W_fh = np.asarray(inputs["W_fh"], np.float32)
    b_fh = np.asarray(inputs["b_fh"], np.float32)
    ops = np.asarray(inputs["ops"], np.int32)
    children = np.asarray(inputs["children"], np.int32)
    child_mask = np.asarray(inputs["child_mask"])

    exp_children, exp_mask = _expected_children()
    if (
        ops.shape[0] != N_NODES
        or not np.array_equal(children, exp_children)
        or not np.array_equal(child_mask.astype(bool), exp_mask)
    ):
        return _reference_numpy(
            emb, W_ioux, b_ioux, W_iouh, b_iouh, W_fx, b_fx, W_fh, b_fh,
            ops, children, child_mask,
        )

    # ---- host prep ----
    x = emb[ops]  # [8191, 256]
    x_heap = x[::-1]  # heap order: topo t = N-1-j
    wcT = np.ascontiguousarray(np.concatenate([W_ioux, W_fx], 0).T)  # [256,1024]
    wiouhT = np.ascontiguousarray(W_iouh.T)
    wfhT = np.ascontiguousarray(W_fh.T)
    ident = np.eye(128, dtype=np.float32)
    b_iou_int = np.ascontiguousarray((b_ioux + b_iouh).reshape(6, 128).T)
    b_iou_leaf = np.ascontiguousarray(
        (b_ioux + W_iouh.sum(1) + b_iouh).reshape(6, 128).T
    )
    b_f_int = np.ascontiguousarray((b_fh + b_fx).reshape(2, 128).T)
    b_f_leaf = np.ascontiguousarray((W_fh.sum(1) + b_fh + b_fx).reshape(2, 128).T)

    common = {
        "wcT": wcT,
        "wiouhT": wiouhT,
        "wfhT": wfhT,
        "ident": ident,
        "b_iou_int": b_iou_int,
        "b_iou_leaf": b_iou_leaf,
        "b_f_int": b_f_int,
        "b_f_leaf": b_f_leaf,
    }
    in_maps = []
    for k in range(8):
        idx = _col_index_for_core(k)
        xT = np.zeros((D, NCOL), np.float32)
        xT[:, :1030] = x_heap[idx[:1030]].T
        in_maps.append({"xT": xT, **common})

    global _LAST_IN_MAPS
    _LAST_IN_MAPS = in_maps
    nc = _get_module()
    res = run_bass_kernel_spmd(nc, in_maps, list(range(8)))
    return res.results[0]["out"][:, 0].reshape(2, 256).astype(np.float32)


_LAST_IN_MAPS = None


# revision 9
# speedup vs baseline: 1.6751x; 1.6751x over previous
"""ChildSumTreeLSTM on a complete binary tree (N=8191), 8-core Trainium2.

Strategy: the heap-ordered complete tree = 7 top nodes + 8 independent
1023-node subtrees. Each NeuronCore gets one subtree (tree-level
parallelism), computes the batched x-projections (emb lookup done on host,
projections as dense matmuls on the PE) and a level-synchronous scan
(leaves -> subtree root) with everything feature-major [256 feats x nodes].
One 16KB AllGather shares the 8 subtree roots; the top 3 levels are
computed redundantly on every core. Output read from core 0.
"""

import numpy as np

import concourse.bass as bass
import concourse.tile as tile
from concourse import mybir
from concourse.bass_utils import run_bass_kernel_spmd

F32 = mybir.dt.float32
BF16 = mybir.dt.bfloat16
AFT = mybir.ActivationFunctionType

N_NODES = 8191
D = 256
M = 256
NCOL = 1152  # 1023 subtree cols + 7 top cols + pad
SUB_LEVELS = 10  # subtree: 512 leaves ... 1 root
USE_F32R = True  # flip to use TF32-like fast fp32 matmuls


def _split_excess_waits(nc, max_waits=1):
    """walrus in this container allows only 1 sync-wait per instruction.

    Tile can attach several; hoist the extras onto injected same-engine NOPs
    immediately preceding the instruction (same blocking semantics)."""
    k = 0
    for f in nc.m.functions:
        for bb in f.blocks:
            out = []
            changed = False
            for ins in bb.instructions:
                si = ins.sync_info
                w = list(si.on_wait) if si and si.on_wait else []
                if len(w) > max_waits:
                    hoist, keep = w[:-max_waits], w[-max_waits:]
                    for sw in hoist:
                        nop = mybir.InstNoOp(name=f"whoist{k}", ins=[], outs=[])
                        k += 1
                        nop.engine = ins.engine
                        nop.sync_info = mybir.SyncInfo(on_wait=[sw], on_update=[])
                        out.append(nop)
                    si.on_wait = keep
                    changed = True
                out.append(ins)
            if changed:
                bb.instructions = out


def _mmcast(ap):
    return ap.bitcast(mybir.dt.float32r) if USE_F32R else ap


def _build_module():
    nc = bass.Bass(num_devices=8)

    xT = nc.dram_tensor("xT", [D, NCOL], BF16, kind="ExternalInput")
    wcT = nc.dram_tensor("wcT", [D, 1024], BF16, kind="ExternalInput")
    wiouhT = nc.dram_tensor("wiouhT", [M, 768], BF16, kind="ExternalInput")
    wfhT = nc.dram_tensor("wfhT", [M, 256], BF16, kind="ExternalInput")
    b_iou_int = nc.dram_tensor("b_iou_int", [128, 6], F32, kind="ExternalInput")
    b_iou_leaf = nc.dram_tensor("b_iou_leaf", [128, 6], F32, kind="ExternalInput")
    b_f_int = nc.dram_tensor("b_f_int", [128, 2], F32, kind="ExternalInput")
    b_f_leaf = nc.dram_tensor("b_f_leaf", [128, 2], F32, kind="ExternalInput")
    out = nc.dram_tensor("out", [512, 1], F32, kind="ExternalOutput")

    agin = nc.dram_tensor("agin", [512, 1], F32)
    agout = nc.dram_tensor("agout", [4096, 1], F32, addr_space="Shared")

    with tile.TileContext(nc) as tc:
        with (
            tc.tile_pool(name="consts", bufs=1) as consts,
            tc.tile_pool(name="tmps", bufs=2) as tmps,
            tc.tile_pool(name="scan_psum", bufs=1, space="PSUM") as spsum,
        ):
            # ---- resident SBUF tensors ----
            sb_xT = []
            for kt in range(2):
                t = consts.tile([128, NCOL], BF16, tag=f"xT{kt}")
                nc.gpsimd.dma_start(out=t[:], in_=xT[128 * kt : 128 * (kt + 1), :])
                sb_xT.append(t)
            sb_wcT = []
            for kt in range(2):
                t = consts.tile([128, 1024], BF16, tag=f"wcT{kt}")
                nc.gpsimd.dma_start(out=t[:], in_=wcT[128 * kt : 128 * (kt + 1), :])
                sb_wcT.append(t)
            sb_wiouhT = []
            for kt in range(2):
                t = consts.tile([128, 768], BF16, tag=f"wiouhT{kt}")
                nc.gpsimd.dma_start(out=t[:], in_=wiouhT[128 * kt : 128 * (kt + 1), :])
                sb_wiouhT.append(t)
            sb_wfhT = []
            for kt in range(2):
                t = consts.tile([128, 256], BF16, tag=f"wfhT{kt}")
                nc.gpsimd.dma_start(out=t[:], in_=wfhT[128 * kt : 128 * (kt + 1), :])
                sb_wfhT.append(t)
            sb_biou_i = consts.tile([128, 6], F32, tag="biou_i")
            nc.gpsimd.dma_start(out=sb_biou_i[:], in_=b_iou_int[:])
            sb_biou_l = consts.tile([128, 6], F32, tag="biou_l")
            nc.gpsimd.dma_start(out=sb_biou_l[:], in_=b_iou_leaf[:])
            sb_bf_i = consts.tile([128, 2], F32, tag="bf_i")
            nc.gpsimd.dma_start(out=sb_bf_i[:], in_=b_f_int[:])
            sb_bf_l = consts.tile([128, 2], F32, tag="bf_l")
            nc.gpsimd.dma_start(out=sb_bf_l[:], in_=b_f_leaf[:])

            # IOUXFX[F][p, c]: feature 128F+p for node col c.
            # F 0..1 = i, 2..3 = o, 4..5 = u, 6..7 = fx  (no biases folded)
            IOUXFX = [consts.tile([128, NCOL], F32, tag=f"iouxfx{F}", name=f"iouxfx{F}") for F in range(8)]
            # c/h state for the subtree, cols = local heap index 0..1022
            C = [consts.tile([128, 1024], F32, tag=f"C{h}", name=f"C{h}") for h in range(2)]
            H = [consts.tile([128, 1024], BF16, tag=f"H{h}", name=f"H{h}") for h in range(2)]

            # ---- phase 1: x-projections  IOUXFX = Wc @ x ----
            psum_tags = [f"iou{F}" for F in range(6)] + ["f0", "f1"]
            pre_i = 0
            if True:
                for F in range(8):
                    for c0, cw in ((0, 512), (512, 512), (1024, NCOL - 1024)):
                        ps = spsum.tile([128, 512], F32, tag=psum_tags[pre_i % 8], name=f"pre_ps{pre_i}")
                        pre_i += 1
                        for kt in range(2):
                            nc.tensor.matmul(
                                ps[:, :cw],
                                sb_wcT[kt][:, 128 * F : 128 * (F + 1)],
                                sb_xT[kt][:, c0 : c0 + cw],
                                start=(kt == 0),
                                stop=(kt == 1),
                            )
                        nc.vector.tensor_copy(IOUXFX[F][:, c0 : c0 + cw], ps[:, :cw])

            # ---- phase 2: leaves (local heap 511..1022 -> cols [511:1023)) ----
            a, b = 511, 1023
            for h in range(2):
                sig_i = tmps.tile([128, 512], F32, tag="sig_i")
                nc.scalar.activation(
                    sig_i[:], IOUXFX[0 + h][:, a:b], AFT.Sigmoid,
                    bias=sb_biou_l[:, 0 + h : 1 + h],
                )
                sig_o = tmps.tile([128, 512], F32, tag="sig_o")
                nc.scalar.activation(
                    sig_o[:], IOUXFX[2 + h][:, a:b], AFT.Sigmoid,
                    bias=sb_biou_l[:, 2 + h : 3 + h],
                )
                tanh_u = tmps.tile([128, 512], F32, tag="tanh_u")
                nc.scalar.activation(
                    tanh_u[:], IOUXFX[4 + h][:, a:b], AFT.Tanh,
                    bias=sb_biou_l[:, 4 + h : 5 + h],
                )
                fc = tmps.tile([128, 512], F32, tag="fc")
                nc.scalar.activation(
                    fc[:], IOUXFX[6 + h][:, a:b], AFT.Sigmoid,
                    bias=sb_bf_l[:, h : h + 1],
                )
                iu = tmps.tile([128, 512], F32, tag="iu")
                nc.vector.tensor_mul(iu[:], sig_i[:], tanh_u[:])
                nc.vector.tensor_add(C[h][:, a:b], iu[:], fc[:])
                tanh_c = tmps.tile([128, 512], F32, tag="tanh_c")
                nc.scalar.activation(tanh_c[:], C[h][:, a:b], AFT.Tanh)
                nc.vector.tensor_mul(H[h][:, a:b], sig_o[:], tanh_c[:])

            # ---- internal level routine (feature-major) ----
            def internal_level(n, ioux_lo, childC, childH, Cout, Hout):
                # childC/childH: per h/kt APs [128, 2n] (child cols, heap order)
                # Cout/Hout: per h APs [128, n]
                hs = []
                for kt in range(2):
                    t = tmps.tile([128, max(n, 1)], BF16, tag="hs")
                    nc.vector.tensor_add(
                        t[:, :n], childH[kt][:, 0::2], childH[kt][:, 1::2]
                    )
                    hs.append(t)
                ps_iou = []
                for F in range(6):
                    ps = spsum.tile([128, 512], F32, tag=f"iou{F}")
                    for kt in range(2):
                        nc.tensor.matmul(
                            ps[:, :n],
                            sb_wiouhT[kt][:, 128 * F : 128 * (F + 1)],
                            hs[kt][:, :n],
                            start=(kt == 0),
                            stop=(kt == 1),
                        )
                    pre = tmps.tile([128, max(n, 1)], F32, tag=f"ioupre{F}", name=f"ioupre{F}_{n}_{ioux_lo}")
                    nc.vector.tensor_add(
                        pre[:, :n], ps[:, :n], IOUXFX[F][:, ioux_lo : ioux_lo + n]
                    )
                    ps_iou.append(pre)
                ps_f = []
                for h in range(2):
                    ps = spsum.tile([128, 512], F32, tag=f"f{h}")
                    for kt in range(2):
                        nc.tensor.matmul(
                            ps[:, : 2 * n],
                            sb_wfhT[kt][:, 128 * h : 128 * (h + 1)],
                            childH[kt],
                            start=(kt == 0),
                            stop=(kt == 1),
                        )
                    # + fx[parent] duplicated onto both child slots
                    fxdup = (
                        IOUXFX[6 + h][:, ioux_lo : ioux_lo + n]
                        .unsqueeze(2)
                        .broadcast_to([128, n, 2])
                    )
                    fpre = tmps.tile([128, max(2 * n, 1)], F32, tag=f"fpre{h}", name=f"fpre{h}_{n}_{ioux_lo}")
                    nc.vector.tensor_add(
                        fpre[:, : 2 * n].rearrange("p (n two) -> p n two", two=2),
                        ps[:, : 2 * n].rearrange("p (n two) -> p n two", two=2),
                        fxdup,
                    )
                    ps_f.append(fpre)
                for h in range(2):
                    sig_i = tmps.tile([128, max(n, 1)], F32, tag="sig_i")
                    nc.scalar.activation(
                        sig_i[:, :n], ps_iou[0 + h][:, :n], AFT.Sigmoid,
                        bias=sb_biou_i[:, 0 + h : 1 + h],
                    )
                    sig_o = tmps.tile([128, max(n, 1)], F32, tag="sig_o")
                    nc.scalar.activation(
                        sig_o[:, :n], ps_iou[2 + h][:, :n], AFT.Sigmoid,
                        bias=sb_biou_i[:, 2 + h : 3 + h],
                    )
                    tanh_u = tmps.tile([128, max(n, 1)], F32, tag="tanh_u")
                    nc.scalar.activation(
                        tanh_u[:, :n], ps_iou[4 + h][:, :n], AFT.Tanh,
                        bias=sb_biou_i[:, 4 + h : 5 + h],
                    )
                    f = tmps.tile([128, max(2 * n, 1)], F32, tag="f")
                    nc.scalar.activation(
                        f[:, : 2 * n], ps_f[h][:, : 2 * n], AFT.Sigmoid,
                        bias=sb_bf_i[:, h : h + 1],
                    )
                    g = tmps.tile([128, max(2 * n, 1)], F32, tag="g")
                    nc.vector.tensor_mul(g[:, : 2 * n], f[:, : 2 * n], childC[h])
                    fc = tmps.tile([128, max(n, 1)], F32, tag="fc")
                    nc.vector.tensor_add(fc[:, :n], g[:, 0 : 2 * n : 2], g[:, 1 : 2 * n : 2])
                    iu = tmps.tile([128, max(n, 1)], F32, tag="iu")
                    nc.vector.tensor_mul(iu[:, :n], sig_i[:, :n], tanh_u[:, :n])
                    nc.vector.tensor_add(Cout[h], iu[:, :n], fc[:, :n])
                    tanh_c = tmps.tile([128, max(n, 1)], F32, tag="tanh_c")
                    nc.scalar.activation(tanh_c[:, :n], Cout[h], AFT.Tanh)
                    nc.vector.tensor_mul(Hout[h], sig_o[:, :n], tanh_c[:, :n])

            # ---- phase 3: subtree internal levels (l = 8 .. 0) ----
            for l in range(8, -1, -1):
                n = 1 << l
                a, b = n - 1, 2 * n - 1
                a2, b2 = 2 * n - 1, 4 * n - 1
                internal_level(
                    n,
                    a,
                    [C[h][:, a2:b2] for h in range(2)],
                    [H[kt][:, a2:b2] for kt in range(2)],
                    [C[h][:, a:b] for h in range(2)],
                    [H[h][:, a:b] for h in range(2)],
                )

            # ---- phase 4: AllGather subtree roots ----
            for h in range(2):
                nc.gpsimd.dma_start(
                    out=agin[128 * h : 128 * (h + 1), :], in_=C[h][:, 0:1]
                )
                hroot32 = tmps.tile([128, 1], F32, tag=f"hroot32_{h}", name=f"hroot32_{h}")
                nc.vector.tensor_copy(hroot32[:], H[h][:, 0:1])
                nc.gpsimd.dma_start(
                    out=agin[256 + 128 * h : 256 + 128 * (h + 1), :], in_=hroot32[:]
                )
            nc.gpsimd.collective_compute(
                "AllGather",
                mybir.AluOpType.bypass,
                replica_groups=[list(range(8))],
                ins=[agin[:]],
                outs=[agout[:]],
            )
            C3 = [consts.tile([128, 8], F32, tag=f"C3_{h}", name=f"C3_{h}") for h in range(2)]
            H3f = [consts.tile([128, 8], F32, tag=f"H3f_{h}", name=f"H3f_{h}") for h in range(2)]
            H3 = [consts.tile([128, 8], BF16, tag=f"H3_{h}", name=f"H3_{h}") for h in range(2)]
            for h in range(2):
                nc.gpsimd.dma_start(
                    out=C3[h][:],
                    in_=bass.AP(tensor=agout, offset=128 * h, ap=[[1, 128], [512, 8]]),
                )
                nc.gpsimd.dma_start(
                    out=H3f[h][:],
                    in_=bass.AP(
                        tensor=agout, offset=256 + 128 * h, ap=[[1, 128], [512, 8]]
                    ),
                )
                nc.vector.tensor_copy(H3[h][:], H3f[h][:])

            # ---- phase 5: top 3 levels (redundant on every core) ----
            C2 = [consts.tile([128, 4], F32, tag=f"C2_{h}", name=f"C2_{h}") for h in range(2)]
            H2 = [consts.tile([128, 4], BF16, tag=f"H2_{h}", name=f"H2_{h}") for h in range(2)]
            internal_level(
                4, 1026,
                [C3[h][:] for h in range(2)],
                [H3[kt][:] for kt in range(2)],
                [C2[h][:] for h in range(2)],
                [H2[h][:] for h in range(2)],
            )
            C1 = [consts.tile([128, 2], F32, tag=f"C1_{h}", name=f"C1_{h}") for h in range(2)]
            H1 = [consts.tile([128, 2], BF16, tag=f"H1_{h}", name=f"H1_{h}") for h in range(2)]
            internal_level(
                2, 1024,
                [C2[h][:] for h in range(2)],
                [H2[kt][:] for kt in range(2)],
                [C1[h][:] for h in range(2)],
                [H1[h][:] for h in range(2)],
            )
            C0 = [consts.tile([128, 1], F32, tag=f"C0_{h}", name=f"C0_{h}") for h in range(2)]
            H0 = [consts.tile([128, 1], F32, tag=f"H0_{h}", name=f"H0_{h}") for h in range(2)]
            internal_level(
                1, 1023,
                [C1[h][:] for h in range(2)],
                [H1[kt][:] for kt in range(2)],
                [C0[h][:] for h in range(2)],
                [H0[h][:] for h in range(2)],
            )
            for h in range(2):
                nc.gpsimd.dma_start(out=out[128 * h : 128 * (h + 1), :], in_=C0[h][:])
                nc.gpsimd.dma_start(
                    out=out[256 + 128 * h : 256 + 128 * (h + 1), :], in_=H0[h][:]
                )
    _split_excess_waits(nc)
    return nc


_NC_CACHE = None


def _get_module():
    global _NC_CACHE
    if _NC_CACHE is None:
        _NC_CACHE = _build_module()
    return _NC_CACHE


def _expected_children():
    j = (N_NODES - 1) - np.arange(N_NODES)
    internal = (2 * j + 1) < N_NODES
    ch0 = (N_NODES - 1) - (2 * j + 1)
    ch1 = (N_NODES - 1) - (2 * j + 2)
    children = np.stack(
        [np.where(internal, ch0, 0), np.where(internal, ch1, 0)], axis=1
    ).astype(np.int32)
    mask = np.stack([internal, internal], axis=1)
    return children, mask


def _reference_numpy(emb, W_ioux, b_ioux, W_iouh, b_iouh, W_fx, b_fx, W_fh, b_fh,
                     ops, children, child_mask):
    # generic fallback (matches reference.py) for unexpected tree structure
    def sigmoid(v):
        return 1.0 / (1.0 + np.exp(-v))

    N = ops.shape[0]
    Md = W_fh.shape[0]
    x = emb[ops]
    iou_x = x @ W_ioux.T + b_ioux
    fx_all = x @ W_fx.T + b_fx
    ones = np.ones((Md,), np.float32)
    leaf_fh = ones @ W_fh.T + b_fh
    maskf = child_mask.astype(np.float32)
    c_arr = np.zeros((N, Md), np.float32)
    h_arr = np.zeros((N, Md), np.float32)
    for t in range(N):
        idx = children[t]
        m = maskf[t][:, None]
        ch_c = c_arr[idx] * m
        ch_h = h_arr[idx] * m
        is_leaf = maskf[t].sum() == 0
        h_sum = ones if is_leaf else ch_h.sum(0)
        iou = iou_x[t] + h_sum @ W_iouh.T + b_iouh
        i, o, u = np.split(iou, 3)
        i, o, u = sigmoid(i), sigmoid(o), np.tanh(u)
        f = sigmoid(ch_h @ W_fh.T + b_fh + fx_all[t])
        fc_int = (f * ch_c).sum(0)
        fc_leaf = sigmoid(leaf_fh + fx_all[t])
        fc = fc_leaf if is_leaf else fc_int
        c = i * u + fc
        h = o * np.tanh(c)
        c_arr[t] = c
        h_arr[t] = h
    return np.stack([c_arr[N - 1], h_arr[N - 1]])


def _col_index_for_core(k):
    # columns 0..1022: subtree-local heap order; 1023..1029: global heap 0..6
    idx = np.zeros(NCOL, np.int64)
    for l in range(SUB_LEVELS):
        n = 1 << l
        lo = n - 1
        g0 = (1 << (3 + l)) - 1 + k * n
        idx[lo : lo + n] = g0 + np.arange(n)
    idx[1023:1030] = np.arange(7)
    return idx


def kernel(**inputs):
    emb = np.asarray(inputs["emb"], np.float32)
    W_ioux = np.asarray(inputs["W_ioux"], np.float32)
    b_ioux = np.asarray(inputs["b_ioux"], np.float32)
    W_iouh = np.asarray(inputs["W_iouh"], np.float32)
    b_iouh = np.asarray(inputs["b_iouh"], np.float32)
    W_fx = np.asarray(inputs["W_fx"], np.float32)
    b_fx = np.asarray(inputs["b_fx"], np.float32)
    W_fh = np.asarray(inputs["W_fh"], np.float32)
    b_fh = np.asarray(inputs["b_fh"], np.float32)
    ops = np.asarray(inputs["ops"], np.int32)
    children = np.asarray(inputs["children"], np.int32)
    child_mask = np.asarray(inputs["child_mask"])

    exp_children, exp_mask = _expected_children()
    if (
        ops.shape[0] != N_NODES
        or not np.array_equal(children, exp_children)
        or not np.array_equal(child_mask.astype(bool), exp_mask)
    ):
        return _reference_numpy(
            emb, W_ioux, b_ioux, W_iouh, b_iouh, W_fx, b_fx, W_fh, b_fh,
            ops, children, child_mask,
        )

    # ---- host prep ----
    x = emb[ops]  # [8191, 256]
    x_heap = x[::-1]  # heap order: topo t = N-1-j
    import ml_dtypes

    bf16 = ml_dtypes.bfloat16
    wcT = np.ascontiguousarray(np.concatenate([W_ioux, W_fx], 0).T).astype(bf16)
    wiouhT = np.ascontiguousarray(W_iouh.T).astype(bf16)
    wfhT = np.ascontiguousarray(W_fh.T).astype(bf16)
    b_iou_int = np.ascontiguousarray((b_ioux + b_iouh).reshape(6, 128).T)
    b_iou_leaf = np.ascontiguousarray(
        (b_ioux + W_iouh.sum(1) + b_iouh).reshape(6, 128).T
    )
    b_f_int = np.ascontiguousarray((b_fh + b_fx).reshape(2, 128).T)
    b_f_leaf = np.ascontiguousarray((W_fh.sum(1) + b_fh + b_fx).reshape(2, 128).T)

    common = {
        "wcT": wcT,
        "wiouhT": wiouhT,
        "wfhT": wfhT,
        "b_iou_int": b_iou_int,
        "b_iou_leaf": b_iou_leaf,
        "b_f_int": b_f_int,
        "b_f_leaf": b_f_leaf,
    }
    in_maps = []
    for k in range(8):
        idx = _col_index_for_core(k)
        xT = np.zeros((D, NCOL), bf16)
        xT[:, :1030] = x_heap[idx[:1030]].T.astype(bf16)
        in_maps.append({"xT": xT, **common})

    global _LAST_IN_MAPS
    _LAST_IN_MAPS = in_maps
    nc = _get_module()
    res = run_bass_kernel_spmd(nc, in_maps, list(range(8)))
    return res.results[0]["out"][:, 0].reshape(2, 256).astype(np.float32)


_LAST_IN_MAPS = None


# revision 10
# speedup vs baseline: 3.0058x; 1.7944x over previous
"""ChildSumTreeLSTM on a complete binary tree (N=8191), 8-core Trainium2.

Strategy: the heap-ordered complete tree = 7 top nodes + 8 independent
1023-node subtrees. Each NeuronCore gets one subtree (tree-level
parallelism), computes the batched x-projections (emb lookup done on host,
projections as dense matmuls on the PE) and a level-synchronous scan
(leaves -> subtree root) with everything feature-major [256 feats x nodes].
One 16KB AllGather shares the 8 subtree roots; the top 3 levels are
computed redundantly on every core. Output read from core 0.
"""

import numpy as np

import concourse.bass as bass
import concourse.tile as tile
from concourse import mybir
from concourse.bass_utils import run_bass_kernel_spmd

F32 = mybir.dt.float32
BF16 = mybir.dt.bfloat16
AFT = mybir.ActivationFunctionType

N_NODES = 8191
D = 256
M = 256
NCOL = 1152  # 1023 subtree cols + 7 top cols + pad
SUB_LEVELS = 10  # subtree: 512 leaves ... 1 root
USE_F32R = True  # flip to use TF32-like fast fp32 matmuls


def _split_excess_waits(nc, max_waits=1):
    """walrus in this container allows only 1 sync-wait per instruction.

    Tile can attach several; hoist the extras onto injected same-engine NOPs
    immediately preceding the instruction (same blocking semantics)."""
    k = 0
    for f in nc.m.functions:
        for bb in f.blocks:
            out = []
            changed = False
            for ins in bb.instructions:
                si = ins.sync_info
                w = list(si.on_wait) if si and si.on_wait else []
                if len(w) > max_waits:
                    hoist, keep = w[:-max_waits], w[-max_waits:]
                    for sw in hoist:
                        nop = mybir.InstNoOp(name=f"whoist{k}", ins=[], outs=[])
                        k += 1
                        nop.engine = ins.engine
                        nop.sync_info = mybir.SyncInfo(on_wait=[sw], on_update=[])
                        out.append(nop)
                    si.on_wait = keep
                    changed = True
                out.append(ins)
            if changed:
                bb.instructions = out


def _mmcast(ap):
    return ap.bitcast(mybir.dt.float32r) if USE_F32R else ap


def _build_module():
    nc = bass.Bass(num_devices=8)

    xT = nc.dram_tensor("xT", [D, NCOL], BF16, kind="ExternalInput")
    wcT = nc.dram_tensor("wcT", [D, 1024], BF16, kind="ExternalInput")
    wiouhT = nc.dram_tensor("wiouhT", [M, 768], BF16, kind="ExternalInput")
    wfhT = nc.dram_tensor("wfhT", [M, 256], BF16, kind="ExternalInput")
    b_iou_int = nc.dram_tensor("b_iou_int", [128, 6], F32, kind="ExternalInput")
    b_iou_leaf = nc.dram_tensor("b_iou_leaf", [128, 6], F32, kind="ExternalInput")
    b_f_int = nc.dram_tensor("b_f_int", [128, 2], F32, kind="ExternalInput")
    b_f_leaf = nc.dram_tensor("b_f_leaf", [128, 2], F32, kind="ExternalInput")
    out = nc.dram_tensor("out", [512, 1], F32, kind="ExternalOutput")


    with tile.TileContext(nc) as tc:
        with (
            tc.tile_pool(name="consts", bufs=1) as consts,
            tc.tile_pool(name="tmps", bufs=2) as tmps,
            tc.tile_pool(name="scan_psum", bufs=1, space="PSUM") as spsum,
        ):
            # ---- resident SBUF tensors ----
            sb_xT = []
            for kt in range(2):
                t = consts.tile([128, NCOL], BF16, tag=f"xT{kt}")
                nc.gpsimd.dma_start(out=t[:], in_=xT[128 * kt : 128 * (kt + 1), :])
                sb_xT.append(t)
            sb_wcT = []
            for kt in range(2):
                t = consts.tile([128, 1024], BF16, tag=f"wcT{kt}")
                nc.gpsimd.dma_start(out=t[:], in_=wcT[128 * kt : 128 * (kt + 1), :])
                sb_wcT.append(t)
            sb_wiouhT = []
            for kt in range(2):
                t = consts.tile([128, 768], BF16, tag=f"wiouhT{kt}")
                nc.gpsimd.dma_start(out=t[:], in_=wiouhT[128 * kt : 128 * (kt + 1), :])
                sb_wiouhT.append(t)
            sb_wfhT = []
            for kt in range(2):
                t = consts.tile([128, 256], BF16, tag=f"wfhT{kt}")
                nc.gpsimd.dma_start(out=t[:], in_=wfhT[128 * kt : 128 * (kt + 1), :])
                sb_wfhT.append(t)
            sb_biou_i = consts.tile([128, 6], F32, tag="biou_i")
            nc.gpsimd.dma_start(out=sb_biou_i[:], in_=b_iou_int[:])
            sb_biou_l = consts.tile([128, 6], F32, tag="biou_l")
            nc.gpsimd.dma_start(out=sb_biou_l[:], in_=b_iou_leaf[:])
            sb_bf_i = consts.tile([128, 2], F32, tag="bf_i")
            nc.gpsimd.dma_start(out=sb_bf_i[:], in_=b_f_int[:])
            sb_bf_l = consts.tile([128, 2], F32, tag="bf_l")
            nc.gpsimd.dma_start(out=sb_bf_l[:], in_=b_f_leaf[:])

            # IOUXFX[F][p, c]: feature 128F+p for node col c.
            # F 0..1 = i, 2..3 = o, 4..5 = u, 6..7 = fx  (no biases folded)
            IOUXFX = [consts.tile([128, NCOL], F32, tag=f"iouxfx{F}", name=f"iouxfx{F}") for F in range(8)]
            # c/h state for the subtree, cols = local heap index 0..1022
            C = [consts.tile([128, 1024], F32, tag=f"C{h}", name=f"C{h}") for h in range(2)]
            H = [consts.tile([128, 1024], BF16, tag=f"H{h}", name=f"H{h}") for h in range(2)]

            # ---- phase 1: x-projections  IOUXFX = Wc @ x ----
            psum_tags = [f"iou{F}" for F in range(6)] + ["f0", "f1"]
            pre_i = 0
            if True:
                for F in range(8):
                    for c0, cw in ((0, 512), (512, 512), (1024, NCOL - 1024)):
                        ps = spsum.tile([128, 512], F32, tag=psum_tags[pre_i % 8], name=f"pre_ps{pre_i}")
                        pre_i += 1
                        for kt in range(2):
                            nc.tensor.matmul(
                                ps[:, :cw],
                                sb_wcT[kt][:, 128 * F : 128 * (F + 1)],
                                sb_xT[kt][:, c0 : c0 + cw],
                                start=(kt == 0),
                                stop=(kt == 1),
                            )
                        nc.vector.tensor_copy(IOUXFX[F][:, c0 : c0 + cw], ps[:, :cw])

            # ---- phase 2: leaves (local heap 511..1022 -> cols [511:1023)) ----
            a, b = 511, 1023
            for h in range(2):
                sig_i = tmps.tile([128, 512], F32, tag="sig_i")
                nc.scalar.activation(
                    sig_i[:], IOUXFX[0 + h][:, a:b], AFT.Sigmoid,
                    bias=sb_biou_l[:, 0 + h : 1 + h],
                )
                sig_o = tmps.tile([128, 512], F32, tag="sig_o")
                nc.scalar.activation(
                    sig_o[:], IOUXFX[2 + h][:, a:b], AFT.Sigmoid,
                    bias=sb_biou_l[:, 2 + h : 3 + h],
                )
                tanh_u = tmps.tile([128, 512], F32, tag="tanh_u")
                nc.scalar.activation(
                    tanh_u[:], IOUXFX[4 + h][:, a:b], AFT.Tanh,
                    bias=sb_biou_l[:, 4 + h : 5 + h],
                )
                fc = tmps.tile([128, 512], F32, tag="fc")
                nc.scalar.activation(
                    fc[:], IOUXFX[6 + h][:, a:b], AFT.Sigmoid,
                    bias=sb_bf_l[:, h : h + 1],
                )
                iu = tmps.tile([128, 512], F32, tag="iu")
                nc.vector.tensor_mul(iu[:], sig_i[:], tanh_u[:])
                nc.vector.tensor_add(C[h][:, a:b], iu[:], fc[:])
                tanh_c = tmps.tile([128, 512], F32, tag="tanh_c")
                nc.scalar.activation(tanh_c[:], C[h][:, a:b], AFT.Tanh)
                nc.vector.tensor_mul(H[h][:, a:b], sig_o[:], tanh_c[:])

            # ---- internal level routine (feature-major) ----
            def internal_level(n, ioux_lo, childC, childH, Cout, Hout):
                # childC/childH: per h/kt APs [128, 2n] (child cols, heap order)
                # Cout/Hout: per h APs [128, n]
                hs = []
                for kt in range(2):
                    t = tmps.tile([128, max(n, 1)], BF16, tag="hs")
                    nc.vector.tensor_add(
                        t[:, :n], childH[kt][:, 0::2], childH[kt][:, 1::2]
                    )
                    hs.append(t)
                ps_iou = []
                for F in range(6):
                    ps = spsum.tile([128, 512], F32, tag=f"iou{F}")
                    for kt in range(2):
                        nc.tensor.matmul(
                            ps[:, :n],
                            sb_wiouhT[kt][:, 128 * F : 128 * (F + 1)],
                            hs[kt][:, :n],
                            start=(kt == 0),
                            stop=(kt == 1),
                        )
                    pre = tmps.tile([128, max(n, 1)], F32, tag=f"ioupre{F}", name=f"ioupre{F}_{n}_{ioux_lo}")
                    nc.vector.tensor_add(
                        pre[:, :n], ps[:, :n], IOUXFX[F][:, ioux_lo : ioux_lo + n]
                    )
                    ps_iou.append(pre)
                ps_f = []
                for h in range(2):
                    ps = spsum.tile([128, 512], F32, tag=f"f{h}")
                    for kt in range(2):
                        nc.tensor.matmul(
                            ps[:, : 2 * n],
                            sb_wfhT[kt][:, 128 * h : 128 * (h + 1)],
                            childH[kt],
                            start=(kt == 0),
                            stop=(kt == 1),
                        )
                    # + fx[parent] duplicated onto both child slots
                    fxdup = (
                        IOUXFX[6 + h][:, ioux_lo : ioux_lo + n]
                        .unsqueeze(2)
                        .broadcast_to([128, n, 2])
                    )
                    fpre = tmps.tile([128, max(2 * n, 1)], F32, tag=f"fpre{h}", name=f"fpre{h}_{n}_{ioux_lo}")
                    nc.vector.tensor_add(
                        fpre[:, : 2 * n].rearrange("p (n two) -> p n two", two=2),
                        ps[:, : 2 * n].rearrange("p (n two) -> p n two", two=2),
                        fxdup,
                    )
                    ps_f.append(fpre)
                for h in range(2):
                    sig_i = tmps.tile([128, max(n, 1)], F32, tag="sig_i")
                    nc.scalar.activation(
                        sig_i[:, :n], ps_iou[0 + h][:, :n], AFT.Sigmoid,
                        bias=sb_biou_i[:, 0 + h : 1 + h],
                    )
                    sig_o = tmps.tile([128, max(n, 1)], F32, tag="sig_o")
                    nc.scalar.activation(
                        sig_o[:, :n], ps_iou[2 + h][:, :n], AFT.Sigmoid,
                        bias=sb_biou_i[:, 2 + h : 3 + h],
                    )
                    tanh_u = tmps.tile([128, max(n, 1)], F32, tag="tanh_u")
                    nc.scalar.activation(
                        tanh_u[:, :n], ps_iou[4 + h][:, :n], AFT.Tanh,
                        bias=sb_biou_i[:, 4 + h : 5 + h],
                    )
                    f = tmps.tile([128, max(2 * n, 1)], F32, tag="f")
                    nc.scalar.activation(
                        f[:, : 2 * n], ps_f[h][:, : 2 * n], AFT.Sigmoid,
                        bias=sb_bf_i[:, h : h + 1],
                    )
                    g = tmps.tile([128, max(2 * n, 1)], F32, tag="g")
                    nc.vector.tensor_mul(g[:, : 2 * n], f[:, : 2 * n], childC[h])
                    fc = tmps.tile([128, max(n, 1)], F32, tag="fc")
                    nc.vector.tensor_add(fc[:, :n], g[:, 0 : 2 * n : 2], g[:, 1 : 2 * n : 2])
                    iu = tmps.tile([128, max(n, 1)], F32, tag="iu")
                    nc.vector.tensor_mul(iu[:, :n], sig_i[:, :n], tanh_u[:, :n])
                    nc.vector.tensor_add(Cout[h], iu[:, :n], fc[:, :n])
                    tanh_c = tmps.tile([128, max(n, 1)], F32, tag="tanh_c")
                    nc.scalar.activation(tanh_c[:, :n], Cout[h], AFT.Tanh)
                    nc.vector.tensor_mul(Hout[h], sig_o[:, :n], tanh_c[:, :n])

            # ---- phase 3: subtree internal levels (l = 8 .. 0) ----
            for l in range(8, -1, -1):
                n = 1 << l
                a, b = n - 1, 2 * n - 1
                a2, b2 = 2 * n - 1, 4 * n - 1
                internal_level(
                    n,
                    a,
                    [C[h][:, a2:b2] for h in range(2)],
                    [H[kt][:, a2:b2] for kt in range(2)],
                    [C[h][:, a:b] for h in range(2)],
                    [H[h][:, a:b] for h in range(2)],
                )

            # ---- phase 4: emit subtree root (c,h); top-7 merge on host ----
            for h in range(2):
                nc.gpsimd.dma_start(
                    out=out[128 * h : 128 * (h + 1), :], in_=C[h][:, 0:1]
                )
                hroot32 = tmps.tile([128, 1], F32, tag=f"hroot32_{h}", name=f"hroot32_{h}")
                nc.vector.tensor_copy(hroot32[:], H[h][:, 0:1])
                nc.gpsimd.dma_start(
                    out=out[256 + 128 * h : 256 + 128 * (h + 1), :], in_=hroot32[:]
                )
    _split_excess_waits(nc)
    return nc


_NC_CACHE = None


def _get_module():
    global _NC_CACHE
    if _NC_CACHE is None:
        _NC_CACHE = _build_module()
    return _NC_CACHE


def _expected_children():
    j = (N_NODES - 1) - np.arange(N_NODES)
    internal = (2 * j + 1) < N_NODES
    ch0 = (N_NODES - 1) - (2 * j + 1)
    ch1 = (N_NODES - 1) - (2 * j + 2)
    children = np.stack(
        [np.where(internal, ch0, 0), np.where(internal, ch1, 0)], axis=1
    ).astype(np.int32)
    mask = np.stack([internal, internal], axis=1)
    return children, mask


def _reference_numpy(emb, W_ioux, b_ioux, W_iouh, b_iouh, W_fx, b_fx, W_fh, b_fh,
                     ops, children, child_mask):
    # generic fallback (matches reference.py) for unexpected tree structure
    def sigmoid(v):
        return 1.0 / (1.0 + np.exp(-v))

    N = ops.shape[0]
    Md = W_fh.shape[0]
    x = emb[ops]
    iou_x = x @ W_ioux.T + b_ioux
    fx_all = x @ W_fx.T + b_fx
    ones = np.ones((Md,), np.float32)
    leaf_fh = ones @ W_fh.T + b_fh
    maskf = child_mask.astype(np.float32)
    c_arr = np.zeros((N, Md), np.float32)
    h_arr = np.zeros((N, Md), np.float32)
    for t in range(N):
        idx = children[t]
        m = maskf[t][:, None]
        ch_c = c_arr[idx] * m
        ch_h = h_arr[idx] * m
        is_leaf = maskf[t].sum() == 0
        h_sum = ones if is_leaf else ch_h.sum(0)
        iou = iou_x[t] + h_sum @ W_iouh.T + b_iouh
        i, o, u = np.split(iou, 3)
        i, o, u = sigmoid(i), sigmoid(o), np.tanh(u)
        f = sigmoid(ch_h @ W_fh.T + b_fh + fx_all[t])
        fc_int = (f * ch_c).sum(0)
        fc_leaf = sigmoid(leaf_fh + fx_all[t])
        fc = fc_leaf if is_leaf else fc_int
        c = i * u + fc
        h = o * np.tanh(c)
        c_arr[t] = c
        h_arr[t] = h
    return np.stack([c_arr[N - 1], h_arr[N - 1]])


def _col_index_for_core(k):
    # columns 0..1022: subtree-local heap order; 1023..1029: global heap 0..6
    idx = np.zeros(NCOL, np.int64)
    for l in range(SUB_LEVELS):
        n = 1 << l
        lo = n - 1
        g0 = (1 << (3 + l)) - 1 + k * n
        idx[lo : lo + n] = g0 + np.arange(n)
    idx[1023:1030] = np.arange(7)
    return idx


def kernel(**inputs):
    emb = np.asarray(inputs["emb"], np.float32)
    W_ioux = np.asarray(inputs["W_ioux"], np.float32)
    b_ioux = np.asarray(inputs["b_ioux"], np.float32)
    W_iouh = np.asarray(inputs["W_iouh"], np.float32)
    b_iouh = np.asarray(inputs["b_iouh"], np.float32)
    W_fx = np.asarray(inputs["W_fx"], np.float32)
    b_fx = np.asarray(inputs["b_fx"], np.float32)
    W_fh = np.asarray(inputs["W_fh"], np.float32)
    b_fh = np.asarray(inputs["b_fh"], np.float32)
    ops = np.asarray(inputs["ops"], np.int32)
    children = np.asarray(inputs["children"], np.int32)
    child_mask = np.asarray(inputs["child_mask"])

    exp_children, exp_mask = _expected_children()
    if (
        ops.shape[0] != N_NODES
        or not np.array_equal(children, exp_children)
        or not np.array_equal(child_mask.astype(bool), exp_mask)
    ):
        return _reference_numpy(
            emb, W_ioux, b_ioux, W_iouh, b_iouh, W_fx, b_fx, W_fh, b_fh,
            ops, children, child_mask,
        )

    # ---- host prep ----
    x = emb[ops]  # [8191, 256]
    x_heap = x[::-1]  # heap order: topo t = N-1-j
    import ml_dtypes

    bf16 = ml_dtypes.bfloat16
    wcT = np.ascontiguousarray(np.concatenate([W_ioux, W_fx], 0).T).astype(bf16)
    wiouhT = np.ascontiguousarray(W_iouh.T).astype(bf16)
    wfhT = np.ascontiguousarray(W_fh.T).astype(bf16)
    b_iou_int = np.ascontiguousarray((b_ioux + b_iouh).reshape(6, 128).T)
    b_iou_leaf = np.ascontiguousarray(
        (b_ioux + W_iouh.sum(1) + b_iouh).reshape(6, 128).T
    )
    b_f_int = np.ascontiguousarray((b_fh + b_fx).reshape(2, 128).T)
    b_f_leaf = np.ascontiguousarray((W_fh.sum(1) + b_fh + b_fx).reshape(2, 128).T)

    common = {
        "wcT": wcT,
        "wiouhT": wiouhT,
        "wfhT": wfhT,
        "b_iou_int": b_iou_int,
        "b_iou_leaf": b_iou_leaf,
        "b_f_int": b_f_int,
        "b_f_leaf": b_f_leaf,
    }
    in_maps = []
    for k in range(8):
        idx = _col_index_for_core(k)
        xT = np.zeros((D, NCOL), bf16)
        xT[:, :1030] = x_heap[idx[:1030]].T.astype(bf16)
        in_maps.append({"xT": xT, **common})

    global _LAST_IN_MAPS
    _LAST_IN_MAPS = in_maps
    nc = _get_module()
    res = run_bass_kernel_spmd(nc, in_maps, list(range(8)))

    # ---- host: merge the 8 subtree roots through the top 7 nodes ----
    def sigmoid(v):
        return 1.0 / (1.0 + np.exp(-v))

    c_arr = np.zeros((15, M), np.float32)
    h_arr = np.zeros((15, M), np.float32)
    for k in range(8):
        r = res.results[k]["out"][:, 0]
        c_arr[7 + k] = r[0:256]
        h_arr[7 + k] = r[256:512]
    x_top = x_heap[0:7].astype(np.float32)
    iou_x7 = x_top @ W_ioux.T + b_ioux
    fx7 = x_top @ W_fx.T + b_fx
    for j in range(6, -1, -1):
        ch = [2 * j + 1, 2 * j + 2]
        h_sum = h_arr[ch[0]] + h_arr[ch[1]]
        iou = iou_x7[j] + h_sum @ W_iouh.T + b_iouh
        i_g, o_g, u_g = np.split(iou, 3)
        i_g, o_g, u_g = sigmoid(i_g), sigmoid(o_g), np.tanh(u_g)
        fc = np.zeros(M, np.float32)
        for cj in ch:
            f = sigmoid(h_arr[cj] @ W_fh.T + b_fh + fx7[j])
            fc += f * c_arr[cj]
        c_arr[j] = i_g * u_g + fc
        h_arr[j] = o_g * np.tanh(c_arr[j])
    return np.stack([c_arr[0], h_arr[0]]).astype(np.float32)


_LAST_IN_MAPS = None
